# revision 29
# baseline (speedup 1.0000x reference)
"""Dual cross-attention (AttentionA) Trainium2 kernel.

Sharding: 8 cores = 4 batches x 2 head-groups (8 heads each).
Per core (batch b, head-group g):
  xn=LN(x_b), xan=LN(xa_b); q,v from xn; ka,va from xan (group's 512 cols)
  per head: S = q ka^T (shared scores), softmax both directions,
  x_upd / xa_upd, partial out-projection with the group's Wout rows.
Host sums the two head-group partials per batch.

Matmuls run as float32r (full-rate fp32 mode, ~1.5e-4 rel err);
layernorm/softmax bookkeeping in fp32. All DMA via SWDGE (gpsimd) --
HWDGE (nc.sync) deadlocks under this runtime config.
"""

import numpy as np

B, N, D = 4, 1024, 1024
P = 128          # partitions
HG = 512         # head-group width per core (8 heads x 64)
HD = 64          # head dim
NT = N // P      # 8 n-tiles
DT = D // P      # 8 d-chunks
CT = HG // P     # 4 c-blocks (head pairs) per group
EPS = 1e-5

_cache = {}


def _build_program(debug=False):
    import concourse.bacc as bacc
    import concourse.mybir as mybir
    from concourse import tile, masks

    F32 = mybir.dt.float32
    F32R = mybir.dt.float32r
    BF16 = mybir.dt.bfloat16
    AF = mybir.ActivationFunctionType
    OP = mybir.AluOpType

    nc = bacc.Bacc("TRN2", target_bir_lowering=False, debug=False, num_devices=1)

    def inp(name, shape):
        return nc.dram_tensor(name, shape, F32, kind="ExternalInput").ap()

    def outp(name, shape):
        return nc.dram_tensor(name, shape, F32, kind="ExternalOutput").ap()

    xb_d = inp("xb", [N, D])
    xab_d = inp("xab", [N, D])
    lnw_d = inp("lnw", [D])
    lnb_d = inp("lnb", [D])
    wq_d = inp("wq", [D, HG])
    wk_d = inp("wk", [D, HG])
    wv_d = inp("wv", [D, HG])
    wo_d = inp("wo", [HG, D])
    ox_d = outp("ox", [N, D])
    oxa_d = outp("oxa", [N, D])
    rx_dram = nc.dram_tensor("rx_scratch", [16, N], F32).ap()
    dbg = {}
    if debug:
        for nm, shp in (("d_xnT", [D, N]), ("d_qT", [HG, N]), ("d_kaT", [HG, N]),
                        ("d_v", [NT * P, HG]), ("d_va", [NT * P, HG]),
                        ("d_e0", [P, N]), ("d_rx0", [P, NT]),
                        ("d_rows", [2, N]), ("d_rbx", [2, N]),
                        ("d_xu", [P, N]), ("d_xau", [P, N])):
            dbg[nm] = outp(nm, shp)

    DMA = nc.gpsimd.dma_start

    with tile.TileContext(nc) as tc:
        with (
            tc.tile_pool(name="persist", bufs=1) as pp,
            tc.tile_pool(name="slabs", bufs=1) as sp,
        ):
            # ---- constants ----
            ident = pp.tile([P, P], F32, tag="ident", name="ident")
            masks.make_identity(nc, ident[:])
            wcol = pp.tile([P, DT], F32, tag="wcol", name="wcol")
            bcolr = pp.tile([P, DT], F32R, tag="bcolr", name="bcolr")
            epsc = pp.tile([P, 1], F32, tag="epsc", name="epsc")
            nc.gpsimd.memset(epsc[:], float(EPS))
            DMA(wcol[:], lnw_d.rearrange("(t p) -> p t", p=P))
            DMA(bcolr[:], lnb_d.rearrange("(t p) -> p t", p=P).bitcast(F32R))
            bcol = bcolr[:].bitcast(F32)

            # ---- persistent slabs (f32r: feed matmuls) ----
            qT = [sp.tile([P, N], F32R, tag=f"qT{t}", name=f"qT{t}") for t in range(CT)]
            kaT = [sp.tile([P, N], F32R, tag=f"kaT{t}", name=f"kaT{t}") for t in range(CT)]
            v_s = [sp.tile([P, HG], BF16, tag=f"v{i}", name=f"v{i}") for i in range(NT)]
            va_s = [sp.tile([P, HG], BF16, tag=f"va{i}", name=f"va{i}") for i in range(NT)]

            # psum helpers: sA/sB = 2-bank tiles, mA..mD = 1-bank tiles
            def ps_s(qpool, tag):
                return qpool.tile([P, 1024], F32, tag=tag, name="ps" + tag)

            def ps_m(qpool, tag):
                return qpool.tile([P, 512], F32, tag=tag, name="pm" + tag)

            with (
                tc.tile_pool(name="xnt", bufs=1) as xp,
                tc.tile_pool(name="work", bufs=1) as wp,
                tc.tile_pool(name="wstream", bufs=3) as wsp,
                tc.tile_pool(name="psum1", bufs=1, space="PSUM") as qq,
            ):
                xnT = [xp.tile([P, N], F32R, tag=f"xnT{j}", name=f"xnT{j}") for j in range(DT)]
                xanT = [xp.tile([P, N], F32R, tag=f"xanT{j}", name=f"xanT{j}") for j in range(DT)]

                # ---- LN + transpose + projections, per tensor ----
                mt4 = ("mA", "mB", "mC", "mD")

                def ln_transpose(src_d, dstT):
                    xh = []
                    for i in range(NT):
                        xt = wp.tile([P, D], F32, tag=f"xt{i}", name=f"xt{i}")
                        DMA(xt[:], src_d[i * P:(i + 1) * P, :])
                        st = wp.tile([P, 12], F32, tag="bnst", name="bnst")
                        nc.vector.bn_stats(st[:, 0:6], xt[:, 0:512])
                        nc.vector.bn_stats(st[:, 6:12], xt[:, 512:1024])
                        ag = wp.tile([P, 2], F32, tag="bnag", name="bnag")
                        nc.vector.bn_aggr(ag[:], st[:])
                        sq = wp.tile([P, 1], F32, tag="sq", name="sq")
                        nc.scalar.activation(sq[:], ag[:, 1:2], AF.Sqrt,
                                             bias=epsc[:])
                        rstd = wp.tile([P, 1], F32, tag="rstd", name="rstd")
                        nc.vector.reciprocal(rstd[:], sq[:])
                        nc.vector.tensor_scalar(xt[:], xt[:], ag[:, 0:1], rstd[:],
                                                op0=OP.subtract, op1=OP.mult)
                        xh.append(xt)
                    for h2 in range(2):
                        sl = slice(h2 * 512, (h2 + 1) * 512)
                        for j in range(DT):
                            pt = ps_m(qq, mt4[(h2 * DT + j) % 4])
                            for io in range(4):
                                i = h2 * 4 + io
                                nc.tensor.transpose(pt[:, io * P:(io + 1) * P],
                                                    xh[i][:, j * P:(j + 1) * P],
                                                    ident[:])
                            nc.scalar.activation(
                                dstT[j][:, sl], pt[:], AF.Identity,
                                bias=bcol[:, j:j + 1],
                                scale=wcol[:, j:j + 1])

                def proj_regions():
                    a, b = ps_s(qq, "sA"), ps_s(qq, "sB")
                    return [a[:, 0:512], a[:, 512:1024],
                            b[:, 0:512], b[:, 512:1024]]

                def proj_qk(w_d, dst, srcT):
                    # 2 waves x 4 groups on sA/sB halves; transposes keep mA-mD
                    for wave in range(2):
                        grp = proj_regions()
                        sel = [(t, nh) for t in range(CT) for nh in range(2)
                               ][wave * 4:wave * 4 + 4]
                        for j in range(DT):
                            wt = wsp.tile([P, HG], F32R, tag="w", name="w")
                            DMA(wt[:], w_d[j * P:(j + 1) * P, :].bitcast(F32R))
                            for g, (t, nh) in enumerate(sel):
                                nc.tensor.matmul(
                                    grp[g], wt[:, t * P:(t + 1) * P],
                                    srcT[j][:, nh * 512:(nh + 1) * 512],
                                    start=(j == 0), stop=(j == DT - 1))
                        for g, (t, nh) in enumerate(sel):
                            nc.scalar.activation(
                                dst[t][:, nh * 512:(nh + 1) * 512], grp[g],
                                AF.Copy)

                def proj_v(w_d, dst, srcT):
                    for wave in range(2):
                        grp = proj_regions()
                        sel = list(range(wave * 4, wave * 4 + 4))
                        for j in range(DT):
                            wt = wsp.tile([P, HG], F32R, tag="w", name="w")
                            DMA(wt[:], w_d[j * P:(j + 1) * P, :].bitcast(F32R))
                            for g, i in enumerate(sel):
                                nc.tensor.matmul(
                                    grp[g],
                                    srcT[j][:, i * P:(i + 1) * P], wt[:],
                                    start=(j == 0), stop=(j == DT - 1))
                        for g, i in enumerate(sel):
                            nc.scalar.activation(dst[i][:], grp[g], AF.Copy)

                ln_transpose(xb_d, xnT)
                proj_qk(wq_d, qT, xnT)
                proj_v(wv_d, v_s, xnT)
                ln_transpose(xab_d, xanT)
                proj_qk(wk_d, kaT, xanT)
                proj_v(wv_d, va_s, xanT)
                if debug:
                    for j in range(DT):
                        DMA(dbg["d_xnT"][j * P:(j + 1) * P, :], xnT[j][:].bitcast(F32))

            # xnt/work/wstream/psum1 released here
            with (
                tc.tile_pool(name="head", bufs=1) as hp_,
                tc.tile_pool(name="expp", bufs=4) as ep,
                tc.tile_pool(name="psum2", bufs=1, space="PSUM") as q2,
            ):
                wo_s = []
                for cc in range(CT):
                    t = hp_.tile([P, D], BF16, tag=f"wo{cc}", name=f"wo{cc}")
                    ws = hp_.tile([P, D], F32, tag="wo_stage", name="wo_stage",
                                  bufs=2)
                    DMA(ws[:], wo_d[cc * P:(cc + 1) * P, :])
                    nc.scalar.activation(t[:], ws[:], AF.Copy)
                    wo_s.append(t)
                xupdT = [hp_.tile([P, N], BF16, tag=f"xu{t}", name=f"xu{t}") for t in range(CT)]
                xaupdT = [hp_.tile([P, N], BF16, tag=f"xau{t}", name=f"xau{t}") for t in range(CT)]

                def recip_rows(rcol0, rcol1, slot, tagp):
                    """[128, 8] per-head rowsum cols -> [128, N] recip bcast."""
                    rr0 = hp_.tile([P, NT], F32, tag="rr0", name="rr0")
                    rr1 = hp_.tile([P, NT], F32, tag="rr1", name="rr1")
                    nc.vector.reciprocal(rr0[:], rcol0[:])
                    nc.vector.reciprocal(rr1[:], rcol1[:])
                    pt = ps_s(q2, "sA")
                    nc.tensor.transpose(pt[0:NT, 0:P], rr0[:], ident[:])
                    nc.tensor.transpose(pt[0:NT, P:2 * P], rr1[:], ident[:])
                    rstage = hp_.tile([NT, 2 * P], F32, tag="rstage", name="rstage")
                    nc.vector.tensor_copy(rstage[:], pt[0:NT, 0:2 * P])
                    DMA(rx_dram[slot:slot + 2, :].rearrange(
                        "r (f p) -> f r p", p=P), rstage[:].rearrange(
                        "f (r p) -> f r p", p=P))
                    rb0 = hp_.tile([P, N], F32, tag=f"{tagp}b0", name=f"{tagp}b0")
                    rb1 = hp_.tile([P, N], F32, tag=f"{tagp}b1", name=f"{tagp}b1")
                    DMA(rb0[:], rx_dram[slot, :].rearrange("n -> () n"
                        ).to_broadcast((P, N)))
                    DMA(rb1[:], rx_dram[slot + 1, :].rearrange("n -> () n"
                        ).to_broadcast((P, N)))
                    return rb0, rb1

                if debug:
                    for t in range(CT):
                        DMA(dbg["d_qT"][t * P:(t + 1) * P, :], qT[t][:].bitcast(F32))
                        DMA(dbg["d_kaT"][t * P:(t + 1) * P, :], kaT[t][:].bitcast(F32))
                    dvst = hp_.tile([P, HG], F32, tag="dvst", name="dvst")
                    for i in range(NT):
                        nc.vector.tensor_copy(dvst[:], v_s[i][:])
                        DMA(dbg["d_v"][i * P:(i + 1) * P, :], dvst[:])
                        nc.vector.tensor_copy(dvst[:], va_s[i][:])
                        DMA(dbg["d_va"][i * P:(i + 1) * P, :], dvst[:])
                pads = {}

                def make_pads(idx):
                    if idx >= CT or idx in pads:
                        return
                    tiles = []
                    for pref, src_t in (("qz", qT[idx]), ("kz", kaT[idx])):
                        for half in range(2):
                            zt = hp_.tile([P, N], F32R, tag=f"{pref}{half}",
                                          name=f"{pref}{half}", bufs=2)
                            d = slice(half * 64, half * 64 + 64)
                            z = slice(64 - half * 64, 128 - half * 64)
                            nc.vector.tensor_copy(zt[d, :], src_t[d, :])
                            nc.gpsimd.memset(zt[z, :].bitcast(F32), 0.0)
                            tiles.append(zt)
                    pads[idx] = (tiles[0], tiles[1], tiles[2], tiles[3])

                make_pads(0)
                for hpi in range(CT):
                    h0c = slice((2 * hpi) * HD % HG, (2 * hpi) * HD % HG + HD)
                    h1c = slice((2 * hpi + 1) * HD % HG,
                                (2 * hpi + 1) * HD % HG + HD)
                    rx0 = hp_.tile([P, NT], F32, tag="rx0", name="rx0")
                    rx1 = hp_.tile([P, NT], F32, tag="rx1", name="rx1")
                    rxa0 = hp_.tile([P, NT], F32, tag="rxa0", name="rxa0")
                    rxa1 = hp_.tile([P, NT], F32, tag="rxa1", name="rxa1")
                    # zero-padded single-head lhsT slabs: full-K matmuls keep
                    # the PE HAM activity monitor warm (K=64 half-array MMs
                    # leave the clock throttled at 1.2 GHz)
                    qz0, qz1, kz0, kz1 = pads.pop(hpi)

                    tga = ("mC", "mD") if hpi % 2 == 0 else ("mA", "mB")
                    tgx = ("mA", "mB") if hpi % 2 == 0 else ("mC", "mD")
                    ps_xa0 = ps_m(q2, tga[0])
                    ps_xa1 = ps_m(q2, tga[1])

                    def upd_mms(psa, psb, lhs_slab, ee0, ee1, idx):
                        for mh, ps in ((0, psa), (1, psb)):
                            nc.tensor.matmul(
                                ps[0:64, :], lhs_slab[idx][:, h0c],
                                ee0[:, mh * 512:(mh + 1) * 512],
                                start=(idx == 0), stop=(idx == NT - 1),
                                tile_position=(0, 0), skip_group_check=True)
                            nc.tensor.matmul(
                                ps[64:128, :], lhs_slab[idx][:, h1c],
                                ee1[:, mh * 512:(mh + 1) * 512],
                                start=(idx == 0), stop=(idx == NT - 1),
                                tile_position=(0, 64), skip_group_check=True)

                    # E phase: S[i] = q ka^T; xa_upd accumulates over n
                    # (upd matmuls deferred one iteration so PE's FIFO queue
                    #  never stalls behind the exp of the current iteration)
                    pend = None
                    for i in range(NT):
                        sa = ps_s(q2, "sA")
                        sb = ps_s(q2, "sB")
                        for mh in range(2):
                            nc.tensor.matmul(
                                sa[:, mh * 512:(mh + 1) * 512],
                                qz0[:, i * P:(i + 1) * P],
                                kaT[hpi][:, mh * 512:(mh + 1) * 512],
                                start=True, stop=True)
                            nc.tensor.matmul(
                                sb[:, mh * 512:(mh + 1) * 512],
                                qz1[:, i * P:(i + 1) * P],
                                kaT[hpi][:, mh * 512:(mh + 1) * 512],
                                start=True, stop=True)
                        e0 = ep.tile([P, N], BF16, tag="E0", name="E0")
                        e1 = ep.tile([P, N], BF16, tag="E1", name="E1")
                        nc.scalar.activation(e0[:], sa[:], AF.Exp,
                                             accum_out=rx0[:, i:i + 1])
                        if debug and hpi == 0 and i == 0:
                            de = hp_.tile([P, N], F32, tag="de", name="de")
                            nc.vector.tensor_copy(de[:], e0[:])
                            DMA(dbg["d_e0"][:], de[:])
                        nc.scalar.activation(e1[:], sb[:], AF.Exp,
                                             accum_out=rx1[:, i:i + 1])
                        if pend is not None:
                            upd_mms(ps_xa0, ps_xa1, v_s, *pend)
                        pend = (e0, e1, i)
                    upd_mms(ps_xa0, ps_xa1, v_s, *pend)

                    if debug and hpi == 0:
                        DMA(dbg["d_rx0"][:], rx0[:])

                    ps_x0 = ps_m(q2, tgx[0])
                    ps_x1 = ps_m(q2, tgx[1])
                    # ET phase: S^T[j]; x_upd accumulates over m
                    pend = None
                    rbx = None
                    for j in range(NT):
                        sa = ps_s(q2, "sA")
                        sb = ps_s(q2, "sB")
                        for nh in range(2):
                            nc.tensor.matmul(
                                sa[:, nh * 512:(nh + 1) * 512],
                                kz0[:, j * P:(j + 1) * P],
                                qT[hpi][:, nh * 512:(nh + 1) * 512],
                                start=True, stop=True)
                            nc.tensor.matmul(
                                sb[:, nh * 512:(nh + 1) * 512],
                                kz1[:, j * P:(j + 1) * P],
                                qT[hpi][:, nh * 512:(nh + 1) * 512],
                                start=True, stop=True)
                        et0 = ep.tile([P, N], BF16, tag="E0", name="E0")
                        et1 = ep.tile([P, N], BF16, tag="E1", name="E1")
                        nc.scalar.activation(et0[:], sa[:], AF.Exp,
                                             accum_out=rxa0[:, j:j + 1])
                        nc.scalar.activation(et1[:], sb[:], AF.Exp,
                                             accum_out=rxa1[:, j:j + 1])
                        if pend is not None:
                            upd_mms(ps_x0, ps_x1, va_s, *pend)
                        pend = (et0, et1, j)
                        if j == 1:
                            # overlap the x-path recip-row chain with ET S-work
                            rbx = recip_rows(rx0, rx1, 4 * hpi, "rx")
                        if j == 3:
                            make_pads(hpi + 1)
                    upd_mms(ps_x0, ps_x1, va_s, *pend)
                    rbx0, rbx1 = rbx

                    rbxa0, rbxa1 = recip_rows(rxa0, rxa1, 4 * hpi + 2, "rxa")

                    for nh, psx in ((0, ps_x0), (1, ps_x1)):
                        sl = slice(nh * 512, (nh + 1) * 512)
                        nc.vector.tensor_tensor(out=xupdT[hpi][0:64, sl],
                                                in0=psx[0:64, :],
                                                in1=rbx0[0:64, sl], op=OP.mult)
                        nc.vector.tensor_tensor(out=xupdT[hpi][64:128, sl],
                                                in0=psx[64:128, :],
                                                in1=rbx1[64:128, sl], op=OP.mult)
                    for mh, psxa in ((0, ps_xa0), (1, ps_xa1)):
                        sl = slice(mh * 512, (mh + 1) * 512)
                        nc.vector.tensor_tensor(out=xaupdT[hpi][0:64, sl],
                                                in0=psxa[0:64, :],
                                                in1=rbxa0[0:64, sl], op=OP.mult)
                        nc.vector.tensor_tensor(out=xaupdT[hpi][64:128, sl],
                                                in0=psxa[64:128, :],
                                                in1=rbxa1[64:128, sl], op=OP.mult)

                if debug:
                    dxu = hp_.tile([P, N], F32, tag="dxu", name="dxu")
                    nc.vector.tensor_copy(dxu[:], xupdT[0][:])
                    DMA(dbg["d_xu"][:], dxu[:])
                    nc.vector.tensor_copy(dxu[:], xaupdT[0][:])
                    DMA(dbg["d_xau"][:], dxu[:])
                # ---- out-projection ----
                mtags = ["mA", "mB", "mC", "mD"]
                k = 0
                for slab, o_d in ((xupdT, ox_d), (xaupdT, oxa_d)):
                    for i in range(NT):
                        ob = hp_.tile([P, 1024], F32, tag="ob", name="ob",
                                      bufs=3)
                        for nh in range(2):
                            g = ps_m(q2, mtags[k % 4]); k += 1
                            for cc in range(CT):
                                nc.tensor.matmul(
                                    g[:], slab[cc][:, i * P:(i + 1) * P],
                                    wo_s[cc][:, nh * 512:(nh + 1) * 512],
                                    start=(cc == 0), stop=(cc == CT - 1))
                            nc.vector.tensor_copy(
                                ob[:, nh * 512:(nh + 1) * 512], g[:])
                        DMA(o_d[i * P:(i + 1) * P, :], ob[:])

    nc.finalize()
    return nc


def _get_program(debug=False):
    key = "ncdbg" if debug else "nc"
    if key not in _cache:
        _cache[key] = _build_program(debug)
    return _cache[key]


def _shard_inputs(x, xa, ln_w, ln_b, Wq, Wkv, Wout):
    x = np.asarray(x, dtype=np.float32)
    xa = np.asarray(xa, dtype=np.float32)
    ln_w = np.ascontiguousarray(np.asarray(ln_w, dtype=np.float32))
    ln_b = np.ascontiguousarray(np.asarray(ln_b, dtype=np.float32))
    Wq = np.asarray(Wq, dtype=np.float32)
    Wkv = np.asarray(Wkv, dtype=np.float32)
    Wout = np.asarray(Wout, dtype=np.float32)
    in_maps = []
    for core in range(8):
        b, g = core // 2, core % 2
        cols = slice(g * HG, (g + 1) * HG)
        in_maps.append({
            "xb": np.ascontiguousarray(x[b]),
            "xab": np.ascontiguousarray(xa[b]),
            "lnw": ln_w,
            "lnb": ln_b,
            "wq": np.ascontiguousarray(Wq[:, cols]),
            "wk": np.ascontiguousarray(Wkv[:, :D][:, cols]),
            "wv": np.ascontiguousarray(Wkv[:, D:][:, cols]),
            "wo": np.ascontiguousarray(Wout[cols, :]),
        })
    return in_maps


def kernel(x, xa, ln_w, ln_b, Wq, Wkv, Wout):
    from concourse.bass_utils import run_bass_kernel_spmd
    nc = _get_program()
    in_maps = _shard_inputs(x, xa, ln_w, ln_b, Wq, Wkv, Wout)
    res = run_bass_kernel_spmd(nc, in_maps, list(range(8)))
    out_x = np.empty((B, N, D), np.float32)
    out_xa = np.empty((B, N, D), np.float32)
    for b in range(B):
        out_x[b] = res.results[2 * b]["ox"] + res.results[2 * b + 1]["ox"]
        out_xa[b] = res.results[2 * b]["oxa"] + res.results[2 * b + 1]["oxa"]
    return out_x, out_xa


# revision 32
# speedup vs baseline: 1.0693x; 1.0693x over previous
"""Dual cross-attention (AttentionA) Trainium2 kernel.

Sharding: 8 cores = 4 batches x 2 head-groups (8 heads each).
Per core (batch b, head-group g):
  xn=LN(x_b), xan=LN(xa_b); q,v from xn; ka,va from xan (group's 512 cols)
  per head: S = q ka^T (shared scores), softmax both directions,
  x_upd / xa_upd, partial out-projection with the group's Wout rows.
Host sums the two head-group partials per batch.

Projections and scores run as float32r (full-rate fp32 PE mode,
~1.5e-4 rel err); the update matmuls and out-projection run in bf16
(f32r cannot target PSUM partitions 64-127, which the column-paired
update matmuls need); layernorm/softmax bookkeeping in fp32. Score
matmuls zero-pad lhsT to K=128: half-array matmuls do not register as
PE-HAM activity and leave the clock throttled at 1.2 GHz. All DMA via
SWDGE (gpsimd) -- HWDGE (nc.sync) deadlocks under this runtime config.
"""

import numpy as np

B, N, D = 4, 1024, 1024
P = 128          # partitions
HG = 512         # head-group width per core (8 heads x 64)
HD = 64          # head dim
NT = N // P      # 8 n-tiles
DT = D // P      # 8 d-chunks
CT = HG // P     # 4 c-blocks (head pairs) per group
EPS = 1e-5

_cache = {}


def _build_program(debug=False):
    import concourse.bacc as bacc
    import concourse.mybir as mybir
    from concourse import tile, masks

    F32 = mybir.dt.float32
    F32R = mybir.dt.float32r
    BF16 = mybir.dt.bfloat16
    AF = mybir.ActivationFunctionType
    OP = mybir.AluOpType

    nc = bacc.Bacc("TRN2", target_bir_lowering=False, debug=False, num_devices=1)

    def inp(name, shape):
        return nc.dram_tensor(name, shape, F32, kind="ExternalInput").ap()

    def outp(name, shape):
        return nc.dram_tensor(name, shape, F32, kind="ExternalOutput").ap()

    xb_d = inp("xb", [N, D])
    xab_d = inp("xab", [N, D])
    lnw_d = inp("lnw", [D])
    lnb_d = inp("lnb", [D])
    wq_d = inp("wq", [D, HG])
    wk_d = inp("wk", [D, HG])
    wv_d = inp("wv", [D, HG])
    wo_d = inp("wo", [HG, D])
    ox_d = outp("ox", [N, D])
    oxa_d = outp("oxa", [N, D])
    rx_dram = nc.dram_tensor("rx_scratch", [16, N], F32).ap()
    dbg = {}
    if debug:
        for nm, shp in (("d_xnT", [D, N]), ("d_qT", [HG, N]), ("d_kaT", [HG, N]),
                        ("d_v", [NT * P, HG]), ("d_va", [NT * P, HG]),
                        ("d_e0", [P, N]), ("d_rx0", [P, NT]),
                        ("d_rows", [2, N]), ("d_rbx", [2, N]),
                        ("d_xu", [P, N]), ("d_xau", [P, N])):
            dbg[nm] = outp(nm, shp)

    DMA = nc.gpsimd.dma_start

    with tile.TileContext(nc) as tc:
        with (
            tc.tile_pool(name="persist", bufs=1) as pp,
            tc.tile_pool(name="slabs", bufs=1) as sp,
        ):
            # ---- constants ----
            ident = pp.tile([P, P], F32, tag="ident", name="ident")
            masks.make_identity(nc, ident[:])
            wcol = pp.tile([P, DT], F32, tag="wcol", name="wcol")
            bcolr = pp.tile([P, DT], F32R, tag="bcolr", name="bcolr")
            DMA(wcol[:], lnw_d.rearrange("(t p) -> p t", p=P))
            DMA(bcolr[:], lnb_d.rearrange("(t p) -> p t", p=P).bitcast(F32R))
            bcol = bcolr[:].bitcast(F32)

            # ---- persistent slabs (f32r: feed matmuls) ----
            qT = [sp.tile([P, N], F32R, tag=f"qT{t}", name=f"qT{t}") for t in range(CT)]
            kaT = [sp.tile([P, N], F32R, tag=f"kaT{t}", name=f"kaT{t}") for t in range(CT)]
            v_s = [sp.tile([P, HG], BF16, tag=f"v{i}", name=f"v{i}") for i in range(NT)]
            va_s = [sp.tile([P, HG], BF16, tag=f"va{i}", name=f"va{i}") for i in range(NT)]

            # psum helpers: sA/sB = 2-bank tiles, mA..mD = 1-bank tiles
            def ps_s(qpool, tag):
                return qpool.tile([P, 1024], F32, tag=tag, name="ps" + tag)

            def ps_m(qpool, tag):
                return qpool.tile([P, 512], F32, tag=tag, name="pm" + tag)

            with (
                tc.tile_pool(name="xnt", bufs=1) as xp,
                tc.tile_pool(name="work", bufs=1) as wp,
                tc.tile_pool(name="wstream", bufs=3) as wsp,
                tc.tile_pool(name="psum1", bufs=1, space="PSUM") as qq,
            ):
                xnT = [xp.tile([P, N], F32R, tag=f"xnT{j}", name=f"xnT{j}") for j in range(DT)]
                xanT = [xp.tile([P, N], F32R, tag=f"xanT{j}", name=f"xanT{j}") for j in range(DT)]

                # ---- LN + transpose + projections, per tensor ----
                mt4 = ("mA", "mB", "mC", "mD")

                def ln_transpose(src_d, dstT):
                    xh = []
                    for i in range(NT):
                        xt = wp.tile([P, D], F32, tag=f"xt{i}", name=f"xt{i}")
                        DMA(xt[:], src_d[i * P:(i + 1) * P, :])
                        st = wp.tile([P, 12], F32, tag="bnst", name="bnst")
                        nc.vector.bn_stats(st[:, 0:6], xt[:, 0:512])
                        nc.vector.bn_stats(st[:, 6:12], xt[:, 512:1024])
                        ag = wp.tile([P, 2], F32, tag="bnag", name="bnag")
                        nc.vector.bn_aggr(ag[:], st[:])
                        veps = wp.tile([P, 1], F32, tag="veps", name="veps")
                        nc.vector.tensor_scalar(veps[:], ag[:, 1:2], float(EPS),
                                                None, op0=OP.add)
                        sq = wp.tile([P, 1], F32, tag="sq", name="sq")
                        nc.scalar.activation(sq[:], veps[:], AF.Sqrt)
                        rstd = wp.tile([P, 1], F32, tag="rstd", name="rstd")
                        nc.vector.reciprocal(rstd[:], sq[:])
                        nc.vector.tensor_scalar(xt[:], xt[:], ag[:, 0:1], rstd[:],
                                                op0=OP.subtract, op1=OP.mult)
                        xh.append(xt)
                    for h2 in range(2):
                        sl = slice(h2 * 512, (h2 + 1) * 512)
                        for j in range(DT):
                            pt = ps_m(qq, mt4[(h2 * DT + j) % 4])
                            for io in range(4):
                                i = h2 * 4 + io
                                nc.tensor.transpose(pt[:, io * P:(io + 1) * P],
                                                    xh[i][:, j * P:(j + 1) * P],
                                                    ident[:])
                            nc.scalar.activation(
                                dstT[j][:, sl], pt[:], AF.Identity,
                                bias=bcol[:, j:j + 1],
                                scale=wcol[:, j:j + 1])

                def proj_regions():
                    a, b = ps_s(qq, "sA"), ps_s(qq, "sB")
                    return [a[:, 0:512], a[:, 512:1024],
                            b[:, 0:512], b[:, 512:1024]]

                def proj_qk(w_d, dst, srcT):
                    # 2 waves x 4 groups on sA/sB halves; transposes keep mA-mD
                    for wave in range(2):
                        grp = proj_regions()
                        sel = [(t, nh) for t in range(CT) for nh in range(2)
                               ][wave * 4:wave * 4 + 4]
                        for j in range(DT):
                            wt = wsp.tile([P, HG], F32R, tag="w", name="w")
                            DMA(wt[:], w_d[j * P:(j + 1) * P, :].bitcast(F32R))
                            for g, (t, nh) in enumerate(sel):
                                nc.tensor.matmul(
                                    grp[g], wt[:, t * P:(t + 1) * P],
                                    srcT[j][:, nh * 512:(nh + 1) * 512],
                                    start=(j == 0), stop=(j == DT - 1))
                        for g, (t, nh) in enumerate(sel):
                            nc.scalar.activation(
                                dst[t][:, nh * 512:(nh + 1) * 512], grp[g],
                                AF.Copy)

                def proj_v(w_d, dst, srcT):
                    for wave in range(2):
                        grp = proj_regions()
                        sel = list(range(wave * 4, wave * 4 + 4))
                        for j in range(DT):
                            wt = wsp.tile([P, HG], F32R, tag="w", name="w")
                            DMA(wt[:], w_d[j * P:(j + 1) * P, :].bitcast(F32R))
                            for g, i in enumerate(sel):
                                nc.tensor.matmul(
                                    grp[g],
                                    srcT[j][:, i * P:(i + 1) * P], wt[:],
                                    start=(j == 0), stop=(j == DT - 1))
                        for g, i in enumerate(sel):
                            nc.scalar.activation(dst[i][:], grp[g], AF.Copy)

                ln_transpose(xb_d, xnT)
                proj_qk(wq_d, qT, xnT)
                proj_v(wv_d, v_s, xnT)
                ln_transpose(xab_d, xanT)
                proj_qk(wk_d, kaT, xanT)
                proj_v(wv_d, va_s, xanT)
                if debug:
                    for j in range(DT):
                        DMA(dbg["d_xnT"][j * P:(j + 1) * P, :], xnT[j][:].bitcast(F32))

            # xnt/work/wstream/psum1 released here
            with (
                tc.tile_pool(name="head", bufs=1) as hp_,
                tc.tile_pool(name="expp", bufs=4) as ep,
                tc.tile_pool(name="psum2", bufs=1, space="PSUM") as q2,
            ):
                wo_s = []
                for cc in range(CT):
                    t = hp_.tile([P, D], BF16, tag=f"wo{cc}", name=f"wo{cc}")
                    ws = hp_.tile([P, D], F32, tag="wo_stage", name="wo_stage",
                                  bufs=2)
                    DMA(ws[:], wo_d[cc * P:(cc + 1) * P, :])
                    nc.scalar.activation(t[:], ws[:], AF.Copy)
                    wo_s.append(t)
                xupdT = [hp_.tile([P, N], BF16, tag=f"xu{t}", name=f"xu{t}") for t in range(CT)]
                xaupdT = [hp_.tile([P, N], BF16, tag=f"xau{t}", name=f"xau{t}") for t in range(CT)]

                def recip_rows(rcol0, rcol1, slot, tagp):
                    """[128, 8] per-head rowsum cols -> [128, N] recip bcast."""
                    rr0 = hp_.tile([P, NT], F32, tag="rr0", name="rr0")
                    rr1 = hp_.tile([P, NT], F32, tag="rr1", name="rr1")
                    nc.vector.reciprocal(rr0[:], rcol0[:])
                    nc.vector.reciprocal(rr1[:], rcol1[:])
                    pt = ps_s(q2, "sA")
                    nc.tensor.transpose(pt[0:NT, 0:P], rr0[:], ident[:])
                    nc.tensor.transpose(pt[0:NT, P:2 * P], rr1[:], ident[:])
                    rstage = hp_.tile([NT, 2 * P], F32, tag="rstage", name="rstage")
                    nc.vector.tensor_copy(rstage[:], pt[0:NT, 0:2 * P])
                    DMA(rx_dram[slot:slot + 2, :].rearrange(
                        "r (f p) -> f r p", p=P), rstage[:].rearrange(
                        "f (r p) -> f r p", p=P))
                    rb0 = hp_.tile([P, N], F32, tag=f"{tagp}b0", name=f"{tagp}b0")
                    rb1 = hp_.tile([P, N], F32, tag=f"{tagp}b1", name=f"{tagp}b1")
                    DMA(rb0[:], rx_dram[slot, :].rearrange("n -> () n"
                        ).to_broadcast((P, N)))
                    DMA(rb1[:], rx_dram[slot + 1, :].rearrange("n -> () n"
                        ).to_broadcast((P, N)))
                    return rb0, rb1

                if debug:
                    for t in range(CT):
                        DMA(dbg["d_qT"][t * P:(t + 1) * P, :], qT[t][:].bitcast(F32))
                        DMA(dbg["d_kaT"][t * P:(t + 1) * P, :], kaT[t][:].bitcast(F32))
                    dvst = hp_.tile([P, HG], F32, tag="dvst", name="dvst")
                    for i in range(NT):
                        nc.vector.tensor_copy(dvst[:], v_s[i][:])
                        DMA(dbg["d_v"][i * P:(i + 1) * P, :], dvst[:])
                        nc.vector.tensor_copy(dvst[:], va_s[i][:])
                        DMA(dbg["d_va"][i * P:(i + 1) * P, :], dvst[:])
                pads = {}

                def make_pads(idx):
                    if idx >= CT or idx in pads:
                        return
                    tiles = []
                    for pref, src_t in (("qz", qT[idx]), ("kz", kaT[idx])):
                        for half in range(2):
                            zt = hp_.tile([P, N], F32R, tag=f"{pref}{half}",
                                          name=f"{pref}{half}", bufs=2)
                            d = slice(half * 64, half * 64 + 64)
                            z = slice(64 - half * 64, 128 - half * 64)
                            nc.vector.tensor_copy(zt[d, :], src_t[d, :])
                            nc.gpsimd.memset(zt[z, :].bitcast(F32), 0.0)
                            tiles.append(zt)
                    pads[idx] = (tiles[0], tiles[1], tiles[2], tiles[3])

                pending_xa = []

                def flush_xa():
                    if not pending_xa:
                        return
                    ph, pr0, pr1, pa0, pa1 = pending_xa.pop()
                    pb0, pb1 = recip_rows(pr0, pr1, 4 * ph + 2, "rxa")
                    for mh, psxa in ((0, pa0), (1, pa1)):
                        sl = slice(mh * 512, (mh + 1) * 512)
                        nc.vector.tensor_tensor(out=xaupdT[ph][0:64, sl],
                                                in0=psxa[0:64, :],
                                                in1=pb0[0:64, sl], op=OP.mult)
                        nc.vector.tensor_tensor(out=xaupdT[ph][64:128, sl],
                                                in0=psxa[64:128, :],
                                                in1=pb1[64:128, sl], op=OP.mult)

                make_pads(0)
                for hpi in range(CT):
                    h0c = slice((2 * hpi) * HD % HG, (2 * hpi) * HD % HG + HD)
                    h1c = slice((2 * hpi + 1) * HD % HG,
                                (2 * hpi + 1) * HD % HG + HD)
                    rx0 = hp_.tile([P, NT], F32, tag="rx0", name="rx0")
                    rx1 = hp_.tile([P, NT], F32, tag="rx1", name="rx1")
                    rxa0 = hp_.tile([P, NT], F32, tag="rxa0", name="rxa0")
                    rxa1 = hp_.tile([P, NT], F32, tag="rxa1", name="rxa1")
                    # zero-padded single-head lhsT slabs: full-K matmuls keep
                    # the PE HAM activity monitor warm (K=64 half-array MMs
                    # leave the clock throttled at 1.2 GHz)
                    qz0, qz1, kz0, kz1 = pads.pop(hpi)

                    tga = ("mC", "mD") if hpi % 2 == 0 else ("mA", "mB")
                    tgx = ("mA", "mB") if hpi % 2 == 0 else ("mC", "mD")
                    ps_xa0 = ps_m(q2, tga[0])
                    ps_xa1 = ps_m(q2, tga[1])

                    def upd_mms(psa, psb, lhs_slab, ee0, ee1, idx):
                        for mh, ps in ((0, psa), (1, psb)):
                            nc.tensor.matmul(
                                ps[0:64, :], lhs_slab[idx][:, h0c],
                                ee0[:, mh * 512:(mh + 1) * 512],
                                start=(idx == 0), stop=(idx == NT - 1),
                                tile_position=(0, 0), skip_group_check=True)
                            nc.tensor.matmul(
                                ps[64:128, :], lhs_slab[idx][:, h1c],
                                ee1[:, mh * 512:(mh + 1) * 512],
                                start=(idx == 0), stop=(idx == NT - 1),
                                tile_position=(0, 64), skip_group_check=True)

                    # E phase: S[i] = q ka^T; xa_upd accumulates over n
                    # (upd matmuls deferred one iteration so PE's FIFO queue
                    #  never stalls behind the exp of the current iteration)
                    pend = None
                    for i in range(NT):
                        sa = ps_s(q2, "sA")
                        sb = ps_s(q2, "sB")
                        for mh in range(2):
                            nc.tensor.matmul(
                                sa[:, mh * 512:(mh + 1) * 512],
                                qz0[:, i * P:(i + 1) * P],
                                kaT[hpi][:, mh * 512:(mh + 1) * 512],
                                start=True, stop=True)
                            nc.tensor.matmul(
                                sb[:, mh * 512:(mh + 1) * 512],
                                qz1[:, i * P:(i + 1) * P],
                                kaT[hpi][:, mh * 512:(mh + 1) * 512],
                                start=True, stop=True)
                        e0 = ep.tile([P, N], BF16, tag="E0", name="E0")
                        e1 = ep.tile([P, N], BF16, tag="E1", name="E1")
                        nc.scalar.activation(e0[:], sa[:], AF.Exp,
                                             accum_out=rx0[:, i:i + 1])
                        if debug and hpi == 0 and i == 0:
                            de = hp_.tile([P, N], F32, tag="de", name="de")
                            nc.vector.tensor_copy(de[:], e0[:])
                            DMA(dbg["d_e0"][:], de[:])
                        nc.scalar.activation(e1[:], sb[:], AF.Exp,
                                             accum_out=rx1[:, i:i + 1])
                        if pend is not None:
                            upd_mms(ps_xa0, ps_xa1, v_s, *pend)
                        pend = (e0, e1, i)
                        if i == 1:
                            flush_xa()
                    upd_mms(ps_xa0, ps_xa1, v_s, *pend)

                    if debug and hpi == 0:
                        DMA(dbg["d_rx0"][:], rx0[:])

                    ps_x0 = ps_m(q2, tgx[0])
                    ps_x1 = ps_m(q2, tgx[1])
                    # ET phase: S^T[j]; x_upd accumulates over m
                    pend = None
                    rbx = None
                    for j in range(NT):
                        sa = ps_s(q2, "sA")
                        sb = ps_s(q2, "sB")
                        for nh in range(2):
                            nc.tensor.matmul(
                                sa[:, nh * 512:(nh + 1) * 512],
                                kz0[:, j * P:(j + 1) * P],
                                qT[hpi][:, nh * 512:(nh + 1) * 512],
                                start=True, stop=True)
                            nc.tensor.matmul(
                                sb[:, nh * 512:(nh + 1) * 512],
                                kz1[:, j * P:(j + 1) * P],
                                qT[hpi][:, nh * 512:(nh + 1) * 512],
                                start=True, stop=True)
                        et0 = ep.tile([P, N], BF16, tag="E0", name="E0")
                        et1 = ep.tile([P, N], BF16, tag="E1", name="E1")
                        nc.scalar.activation(et0[:], sa[:], AF.Exp,
                                             accum_out=rxa0[:, j:j + 1])
                        nc.scalar.activation(et1[:], sb[:], AF.Exp,
                                             accum_out=rxa1[:, j:j + 1])
                        if pend is not None:
                            upd_mms(ps_x0, ps_x1, va_s, *pend)
                        pend = (et0, et1, j)
                        if j == 1:
                            # overlap the x-path recip-row chain with ET S-work
                            rbx = recip_rows(rx0, rx1, 4 * hpi, "rx")
                        if j == 3:
                            make_pads(hpi + 1)
                    upd_mms(ps_x0, ps_x1, va_s, *pend)
                    rbx0, rbx1 = rbx

                    for nh, psx in ((0, ps_x0), (1, ps_x1)):
                        sl = slice(nh * 512, (nh + 1) * 512)
                        nc.vector.tensor_tensor(out=xupdT[hpi][0:64, sl],
                                                in0=psx[0:64, :],
                                                in1=rbx0[0:64, sl], op=OP.mult)
                        nc.vector.tensor_tensor(out=xupdT[hpi][64:128, sl],
                                                in0=psx[64:128, :],
                                                in1=rbx1[64:128, sl], op=OP.mult)
                    pending_xa.append((hpi, rxa0, rxa1, ps_xa0, ps_xa1))

                if debug:
                    dxu = hp_.tile([P, N], F32, tag="dxu", name="dxu")
                    nc.vector.tensor_copy(dxu[:], xupdT[0][:])
                    DMA(dbg["d_xu"][:], dxu[:])
                    nc.vector.tensor_copy(dxu[:], xaupdT[0][:])
                    DMA(dbg["d_xau"][:], dxu[:])
                flush_xa()

                # ---- out-projection ----
                mtags = ["mA", "mB", "mC", "mD"]
                k = 0
                for slab, o_d in ((xupdT, ox_d), (xaupdT, oxa_d)):
                    for i in range(NT):
                        ob = hp_.tile([P, 1024], F32, tag="ob", name="ob",
                                      bufs=3)
                        for nh in range(2):
                            g = ps_m(q2, mtags[k % 4]); k += 1
                            for cc in range(CT):
                                nc.tensor.matmul(
                                    g[:], slab[cc][:, i * P:(i + 1) * P],
                                    wo_s[cc][:, nh * 512:(nh + 1) * 512],
                                    start=(cc == 0), stop=(cc == CT - 1))
                            nc.vector.tensor_copy(
                                ob[:, nh * 512:(nh + 1) * 512], g[:])
                        DMA(o_d[i * P:(i + 1) * P, :], ob[:])

    nc.finalize()
    return nc


def _get_program(debug=False):
    key = "ncdbg" if debug else "nc"
    if key not in _cache:
        _cache[key] = _build_program(debug)
    return _cache[key]


def _shard_inputs(x, xa, ln_w, ln_b, Wq, Wkv, Wout):
    x = np.asarray(x, dtype=np.float32)
    xa = np.asarray(xa, dtype=np.float32)
    ln_w = np.ascontiguousarray(np.asarray(ln_w, dtype=np.float32))
    ln_b = np.ascontiguousarray(np.asarray(ln_b, dtype=np.float32))
    Wq = np.asarray(Wq, dtype=np.float32)
    Wkv = np.asarray(Wkv, dtype=np.float32)
    Wout = np.asarray(Wout, dtype=np.float32)
    in_maps = []
    for core in range(8):
        b, g = core // 2, core % 2
        cols = slice(g * HG, (g + 1) * HG)
        in_maps.append({
            "xb": np.ascontiguousarray(x[b]),
            "xab": np.ascontiguousarray(xa[b]),
            "lnw": ln_w,
            "lnb": ln_b,
            "wq": np.ascontiguousarray(Wq[:, cols]),
            "wk": np.ascontiguousarray(Wkv[:, :D][:, cols]),
            "wv": np.ascontiguousarray(Wkv[:, D:][:, cols]),
            "wo": np.ascontiguousarray(Wout[cols, :]),
        })
    return in_maps


def kernel(x, xa, ln_w, ln_b, Wq, Wkv, Wout):
    from concourse.bass_utils import run_bass_kernel_spmd
    nc = _get_program()
    in_maps = _shard_inputs(x, xa, ln_w, ln_b, Wq, Wkv, Wout)
    res = run_bass_kernel_spmd(nc, in_maps, list(range(8)))
    out_x = np.empty((B, N, D), np.float32)
    out_xa = np.empty((B, N, D), np.float32)
    for b in range(B):
        out_x[b] = res.results[2 * b]["ox"] + res.results[2 * b + 1]["ox"]
        out_xa[b] = res.results[2 * b]["oxa"] + res.results[2 * b + 1]["oxa"]
    return out_x, out_xa
